# revision 11
# baseline (speedup 1.0000x reference)
"""Deformable 3D conv (offset-predicting conv + trilinear-sampled 3x3x3 deform conv)
on 8 TRN2 NeuronCores.

Strategy (v4): shard the output D axis (4 planes/core). Per core:
  1. Offset conv as 9 K=96 fp16 matmuls per v-tile (3 pre-shifted input copies
     fold the kw taps into the K dim); split 4+4 vtiles so quarter-0 p-pipeline
     starts early.
  2. p-pipeline on DVE in 4 quarters, software-pipelined one quarter ahead of
     the gather loop: clip, floor, fracs, indices, 8 trilinear corner weights.
  3. Corner-block table (512B row per padded voxel: 8 corners x 32ch fp16)
     built on HOST, passed as a DRAM input.
  4. Wrapped int16 index tensor built via PE select-matmuls (partition
     16g+q -> q with 8x replication) + strided DVE PSUM->SBUF casts.
  5. Batched gather: 32 dma_gather calls (3456 rows each); Q7 descriptor
     generation on Pool is the critical resource. Enlarged SWDGE carveout to
     keep the descriptor ring from stalling generation.
  6. Weighted corner sum on DVE (one fp16 mult + 3 tree adds per chunk);
     weight replication across channels on the scalar (ACT) engine.
  7. Contraction over (n, c) as 7 accumulated PE matmuls -> out[64, 128]/chunk.
"""
import os
import sys

for _p in ('/opt/trn_rl_repo', '/root/.axon_site/_ro/trn_rl_repo'):
    if os.path.isdir(_p) and _p not in sys.path:
        sys.path.insert(0, _p)

import numpy as np
import ml_dtypes  # noqa

import concourse.bass as bass
import concourse.mybir as mybir
import concourse.tile as tile
from concourse import bacc
from concourse.bass_utils import run_bass_kernel_spmd
from concourse.masks import make_identity

F32 = mybir.dt.float32
F16 = mybir.dt.float16
I32 = mybir.dt.int32
I16 = mybir.dt.int16
AL = mybir.AluOpType

# ---------------- problem constants ----------------
C = 32          # input channels
O = 64          # output channels
NN = 27         # kernel sample points
NCORES = 8
DSH = 4         # output d-planes per core
V = DSH * 32 * 32   # voxels per core = 4096
P35 = 35
PL = 16         # extended d-planes per core
PLSZ = P35 * P35    # 1225
XE_ROWS = PL * PLSZ  # 19600
TROWS = XE_ROWS      # table rows (one per padded voxel)
XR_FREE = 19616      # shifted-copy free size (19600 + shift pad)
NVC = 32             # v-chunks of 128
NQ = 4               # p-pipeline quarters
QC = NVC // NQ       # chunks per quarter = 8
GI = NN * 128        # indices per gather (one chunk) = 3456

_PROGRAM_CACHE = {}


def _build_program():
    nc = bacc.Bacc("TRN2", target_bir_lowering=False, debug=False,
                   dynamic_dma_scratch_size=32768,
                   num_swdge_queues=2)

    xr_d = nc.dram_tensor("xr", [96, XR_FREE], F16, kind="ExternalInput").ap()
    tbl_d = nc.dram_tensor("tbl", [TROWS, 256], F16, kind="ExternalInput").ap()
    pc_d = nc.dram_tensor("pc", [128, NVC * 96], F32, kind="ExternalInput").ap()
    dclip_d = nc.dram_tensor("dclip", [128, 2], F32, kind="ExternalInput").ap()
    wp_d = nc.dram_tensor("wp", [96, 9 * 96], F16, kind="ExternalInput").ap()
    wd_d = nc.dram_tensor("wd", [128, 7 * O], F16, kind="ExternalInput").ap()
    sel_d = nc.dram_tensor("sel", [128, 8 * 128], F32, kind="ExternalInput").ap()
    out_d = nc.dram_tensor("out_sl", [O, V], F32, kind="ExternalOutput").ap()

    with tile.TileContext(nc) as tc:
        with tc.tile_pool(name="const", bufs=1) as cpool, \
             tc.tile_pool(name="idxps", bufs=2, space="PSUM") as ips:
            ident = cpool.tile([128, 128], F32)
            make_identity(nc, ident[:])
            wd_sb = cpool.tile([128, 7 * O], F16)
            nc.sync.dma_start(wd_sb[:], wd_d)
            sel_sb = cpool.tile([128, 8 * 128], F32)
            nc.sync.dma_start(sel_sb[:], sel_d)
            # wrapped int16 gather indices: [q+16r, s*216 + n*8 + g] =
            # idx(voxel s*128+16g+q, tap n), replicated over r
            idxw = cpool.tile([128, NVC * 216], I16)
            # trilinear corner weights: [p, (vc*27) * 8]
            wt8 = cpool.tile([128, NVC * 216], F16)
            # persistent p-pipeline tensors
            p_t = cpool.tile([128, NVC * 96], F32)
            q0i = cpool.tile([128, NVC * 96], I32)
            q0f = cpool.tile([128, NVC * 96], F32)
            fd = cpool.tile([128, NVC * 27], F16)
            fh = cpool.tile([128, NVC * 27], F16)
            fw = cpool.tile([128, NVC * 27], F16)
            idxf = cpool.tile([128, NVC * 27], F32)
            wdd = cpool.tile([128, NVC * 27 * 2], F16)
            whh = cpool.tile([128, NVC * 27 * 2], F16)
            www = cpool.tile([128, NVC * 27 * 2], F16)
            t4 = cpool.tile([128, NVC * 27 * 4], F16)
            dclip_sb = cpool.tile([128, 2], F32)
            nc.sync.dma_start(dclip_sb[:], dclip_d)

            pvq = p_t[:].rearrange("p (v x) -> p v x", x=96)
            qvq = q0f[:].rearrange("p (v x) -> p v x", x=96)
            qiq = q0i[:].rearrange("p (v x) -> p v x", x=96)
            iwv = idxw[:].rearrange("p (s n g) -> p s n g", n=NN, g=8)

            def pipe_quarter(q):
                s0, s1 = q * QC, (q + 1) * QC
                dv = pvq[:, s0:s1, 0:27]
                hwv = pvq[:, s0:s1, 32:91]
                nc.vector.scalar_tensor_tensor(
                    out=dv, in0=dv, scalar=dclip_sb[:, 0:1],
                    in1=dclip_sb[:, 1:2].rearrange(
                        "p (a b) -> p a b", b=1).to_broadcast((128, QC, 27)),
                    op0=AL.max, op1=AL.min)
                nc.vector.tensor_scalar(
                    out=hwv, in0=hwv, scalar1=0.0, scalar2=33.0,
                    op0=AL.max, op1=AL.min)

                cl, ch_ = s0 * 96, s1 * 96
                nc.vector.tensor_copy(q0i[:, cl:ch_], p_t[:, cl:ch_])
                nc.vector.tensor_copy(q0f[:, cl:ch_], q0i[:, cl:ch_])
                # guard against round-to-nearest casts: q0f -= (q0f > p)
                # (reuse q0i's storage for the guard mask)
                nc.vector.tensor_tensor(
                    out=qiq[:, s0:s1, :].bitcast(F32),
                    in0=qvq[:, s0:s1, :], in1=pvq[:, s0:s1, :], op=AL.is_gt)
                nc.vector.tensor_sub(
                    qvq[:, s0:s1, :], qvq[:, s0:s1, :],
                    qiq[:, s0:s1, :].bitcast(F32))

                # fracs (fp16) per axis
                fl, fh_ = s0 * 27, s1 * 27
                fdv = fd[:, fl:fh_].rearrange("p (v x) -> p v x", x=27)
                fhv = fh[:, fl:fh_].rearrange("p (v x) -> p v x", x=27)
                fwv = fw[:, fl:fh_].rearrange("p (v x) -> p v x", x=27)
                nc.vector.tensor_sub(fdv, pvq[:, s0:s1, 0:27], qvq[:, s0:s1, 0:27])
                nc.vector.tensor_sub(fhv, pvq[:, s0:s1, 32:59], qvq[:, s0:s1, 32:59])
                nc.vector.tensor_sub(fwv, pvq[:, s0:s1, 64:91], qvq[:, s0:s1, 64:91])

                # d-axis safety clamp to [0, 14]
                q0dv = qvq[:, s0:s1, 0:27]
                nc.vector.tensor_scalar(
                    out=q0dv, in0=q0dv, scalar1=0.0, scalar2=14.0,
                    op0=AL.max, op1=AL.min)

                # idx = (q0d*35 + q0h)*35 + q0w
                iv = idxf[:, fl:fh_].rearrange("p (v x) -> p v x", x=27)
                nc.vector.scalar_tensor_tensor(
                    out=iv, in0=q0dv, scalar=35.0, in1=qvq[:, s0:s1, 32:59],
                    op0=AL.mult, op1=AL.add)
                nc.vector.scalar_tensor_tensor(
                    out=iv, in0=iv, scalar=35.0, in1=qvq[:, s0:s1, 64:91],
                    op0=AL.mult, op1=AL.add)

                # corner weights wt8 for this quarter
                for pair_t, frac_t in ((wdd, fd), (whh, fh), (www, fw)):
                    pvw = pair_t[:, fl * 2:fh_ * 2].rearrange(
                        "p (i e) -> p i e", e=2)
                    fv1 = frac_t[:, fl:fh_].rearrange("p (i o) -> p i o", o=1)
                    nc.vector.tensor_scalar(
                        out=pvw[:, :, 0:1], in0=fv1, scalar1=-1.0, scalar2=1.0,
                        op0=AL.mult, op1=AL.add)
                    nc.vector.tensor_copy(pvw[:, :, 1:2], fv1)
                t4v = t4[:, fl * 4:fh_ * 4].rearrange(
                    "p (i a b) -> p i a b", a=2, b=2)
                nc.vector.tensor_tensor(
                    out=t4v,
                    in0=whh[:, fl * 2:fh_ * 2].rearrange(
                        "p (i a b) -> p i a b", a=2, b=1
                    ).to_broadcast((128, QC * 27, 2, 2)),
                    in1=www[:, fl * 2:fh_ * 2].rearrange(
                        "p (i a b) -> p i a b", a=1, b=2
                    ).to_broadcast((128, QC * 27, 2, 2)),
                    op=AL.mult)
                w8v = wt8[:, fl * 8:fh_ * 8].rearrange(
                    "p (i a b) -> p i a b", a=2, b=4)
                nc.vector.tensor_tensor(
                    out=w8v,
                    in0=wdd[:, fl * 2:fh_ * 2].rearrange(
                        "p (i a b) -> p i a b", a=2, b=1
                    ).to_broadcast((128, QC * 27, 2, 4)),
                    in1=t4[:, fl * 4:fh_ * 4].rearrange(
                        "p (i a b) -> p i a b", a=1, b=4
                    ).to_broadcast((128, QC * 27, 2, 4)),
                    op=AL.mult)

                # wrapped idx via PE select-matmuls + strided cast copies
                for g in range(8):
                    pidx = ips.tile([128, QC * 27], F32, tag="idxps")
                    nc.tensor.matmul(
                        pidx[:, :],
                        lhsT=sel_sb[:, g * 128:(g + 1) * 128],
                        rhs=idxf[:, fl:fh_],
                        start=True, stop=True)
                    nc.vector.tensor_copy(
                        iwv[:, s0:s1, :, g],
                        pidx[:, :].rearrange("p (s n) -> p s n", n=NN))

            # ---------- front: offset conv + transpose ----------
            with tc.tile_pool(name="front", bufs=1) as fpool:
                xr_sb = fpool.tile([96, XR_FREE], F16)
                nc.sync.dma_start(xr_sb[:], xr_d)
                wp_sb = fpool.tile([96, 9 * 96], F16)
                nc.sync.dma_start(wp_sb[:], wp_d)
                pc_sb = fpool.tile([128, NVC * 96], F32)
                nc.sync.dma_start(pc_sb[:], pc_d)
                off_sb = fpool.tile([96, V], F32)

                def conv_vtiles(vts, cps):
                    for vt in vts:
                        dl, hh = vt // 2, vt % 2
                        psc = cps.tile([96, 512], F32, tag="convps")
                        for gid in range(9):
                            kd, kh = gid // 3, gid % 3
                            b0 = (dl + kd + 5) * PLSZ + (hh * 16 + kh) * P35
                            rhs = xr_sb[:, b0:b0 + 16 * P35].rearrange(
                                "p (a b) -> p a b", b=P35)[:, :, 0:32]
                            nc.tensor.matmul(
                                psc[:, :],
                                lhsT=wp_sb[:, gid * 96:(gid + 1) * 96],
                                rhs=rhs,
                                start=(gid == 0),
                                stop=(gid == 8),
                            )
                        nc.scalar.copy(off_sb[:, vt * 512:(vt + 1) * 512], psc[:, :])

                def transpose_chunks(chs, tps):
                    for ch in chs:
                        ptp = tps.tile([128, 96], F32, tag="trps")
                        nc.tensor.transpose(
                            ptp[:, :],
                            off_sb[:, ch * 128:(ch + 1) * 128],
                            ident[0:96, 0:96],
                        )
                        nc.vector.tensor_add(
                            p_t[:, ch * 96:(ch + 1) * 96], ptp[:, :],
                            pc_sb[:, ch * 96:(ch + 1) * 96])

                with tc.tile_pool(name="convps", bufs=2, space="PSUM") as cps, \
                     tc.tile_pool(name="trps", bufs=2, space="PSUM") as tps:
                    conv_vtiles(range(0, 4), cps)
                    transpose_chunks(range(0, 8), tps)
                    pipe_quarter(0)
                    conv_vtiles(range(4, 8), cps)
                    transpose_chunks(range(8, 32), tps)
                    pipe_quarter(1)

            # ---------- gather + lerp + contract, one quarter ahead ----------
            with (
                tc.tile_pool(name="gat", bufs=3) as gpool,
                tc.tile_pool(name="wrep", bufs=2) as wpool,
                tc.tile_pool(name="accp", bufs=4) as apool,
                tc.tile_pool(name="acctp", bufs=4) as t2pool,
                tc.tile_pool(name="ops", bufs=4, space="PSUM") as ops,
                tc.tile_pool(name="outp", bufs=4) as opool,
            ):
                for q in range(NQ):
                    if q + 2 < NQ:
                        pipe_quarter(q + 2)
                    for s in range(q * QC, (q + 1) * QC):
                        rt = gpool.tile([128, NN * 256], F16, tag="rt")
                        nc.gpsimd.dma_gather(
                            rt[:].rearrange("p (i x) -> p i x", x=256),
                            tbl_d,
                            idxw[:, s * 216:(s + 1) * 216],
                            GI,
                            GI,
                            256,
                            single_packet=False,
                            queue_num=s % 2,
                        )
                        wt8r = wpool.tile([128, NN * 256], F16, tag="wt8r")
                        nc.scalar.copy(
                            wt8r[:].rearrange("p (i e c) -> p i e c", e=8, c=32),
                            wt8[:, s * 216:(s + 1) * 216].rearrange(
                                "p (i e o) -> p i e o", e=8, o=1
                            ).to_broadcast((128, NN, 8, 32)))

                        # weighted corner sum
                        nc.vector.tensor_tensor(
                            out=rt[:], in0=rt[:], in1=wt8r[:], op=AL.mult)
                        rv = rt[:].rearrange("p (i x) -> p i x", x=256)
                        nc.vector.tensor_add(
                            rv[:, :, 0:128], rv[:, :, 0:128], rv[:, :, 128:256])
                        nc.vector.tensor_add(
                            rv[:, :, 0:64], rv[:, :, 0:64], rv[:, :, 64:128])
                        acc = apool.tile([128, 896], F16, tag="acc")
                        nc.vector.tensor_tensor(
                            out=acc[:, 0:864].rearrange("p (n c) -> p n c", c=32),
                            in0=rv[:, :, 0:32],
                            in1=rv[:, :, 32:64],
                            op=AL.add)
                        nc.vector.memset(acc[:, 864:896], 0.0)

                        # transpose + contract + write out
                        acct = t2pool.tile([128, 7, 128], F16, tag="acct")
                        nc.sync.dma_start_transpose(out=acct[:], in_=acc[:])
                        pso = ops.tile([64, 128], F32, tag="pso")
                        for kt in range(7):
                            nc.tensor.matmul(
                                pso[:, :],
                                lhsT=wd_sb[:, kt * O:(kt + 1) * O],
                                rhs=acct[:, kt, :],
                                start=(kt == 0), stop=(kt == 6))
                        osb = opool.tile([64, 128], F32, tag="osb")
                        nc.scalar.copy(osb[:], pso[:, :])
                        nc.sync.dma_start(
                            out=out_d[:, s * 128:(s + 1) * 128], in_=osb[:])

    nc.compile()
    return nc


def _host_prep(x, w_p, b_p, w_d):
    """Build per-core input maps."""
    x = np.asarray(x, np.float32)
    w_p = np.asarray(w_p, np.float32)
    b_p = np.asarray(b_p, np.float32)
    w_d = np.asarray(w_d, np.float32)

    # global padded/extended volume, channel-first, fp16:
    # XG[c, g, h', w'] with g = xp_plane + 5 (xp planes -5..39), h', w' in [0,35)
    XG = np.zeros((C, 45, P35, P35), np.float16)
    XG[:, 6:38, 1:33, 1:33] = x[0].astype(np.float16)

    # pc (shared): [128, 32*96] f32
    v = np.arange(V)
    dl, hh, wl = v >> 10, (v >> 5) & 31, v & 31
    r = np.array([-1.0, 0.0, 1.0], np.float32)
    pn_d, pn_h, pn_w = np.meshgrid(r, r, r, indexing='ij')
    pn = np.stack([pn_d.ravel(), pn_h.ravel(), pn_w.ravel()])  # (3, 27)
    pc = np.zeros((V, 96), np.float32)
    pc[:, 0:27] = (dl[:, None] + 6.0) + pn[0][None, :] + b_p[None, 0:27]
    pc[:, 32:59] = (hh[:, None] + 1.0) + pn[1][None, :] + b_p[None, 27:54]
    pc[:, 64:91] = (wl[:, None] + 1.0) + pn[2][None, :] + b_p[None, 54:81]
    pc_t = pc.reshape(NVC, 128, 96).transpose(1, 0, 2).reshape(128, NVC * 96)
    pc_t = np.ascontiguousarray(pc_t, np.float32)

    # wp lhsT: [96, 9*96] fp16; K-row (j, c) = tap (kd, kh, kw=j)
    wp_l = np.zeros((96, 9 * 96), np.float16)
    colmap = np.full(96, -1, np.int64)
    colmap[0:27] = np.arange(27)
    colmap[32:59] = 27 + np.arange(27)
    colmap[64:91] = 54 + np.arange(27)
    for gid in range(9):
        kd, kh = gid // 3, gid % 3
        for m in range(96):
            ch = colmap[m]
            if ch < 0:
                continue
            for j in range(3):
                wp_l[32 * j:32 * (j + 1), gid * 96 + m] = w_p[ch, :, kd, kh, j]

    # wd lhsT: [128, 7*64] fp16 (K-row (g, pk): n = 4g + pk//32, c = pk%32)
    wd_l = np.zeros((128, 7 * O), np.float16)
    for g in range(7):
        for pk in range(128):
            n = 4 * g + pk // 32
            if n >= NN:
                continue
            wd_l[pk, g * O:(g + 1) * O] = w_d[:, pk % 32, n // 9, (n // 3) % 3, n % 3]

    # sel: [128, 8*128] f32; sel[p, g*128 + m] = (p == 16g + m%16)
    sel = np.zeros((128, 8 * 128), np.float32)
    for g in range(8):
        for m in range(128):
            sel[16 * g + (m % 16), g * 128 + m] = 1.0

    in_maps = []
    for k in range(NCORES):
        dlo = 4 * k - 5
        slab = XG[:, 4 * k:4 * k + PL].reshape(C, XE_ROWS)
        xr = np.zeros((96, XR_FREE), np.float16)
        for j in range(3):
            xr[32 * j:32 * (j + 1), 0:XE_ROWS - j] = slab[:, j:]
        # corner-block table: row (d*35+h)*35+w -> [8 corners x 32 ch] fp16
        sp = np.zeros((C, PL + 1, 36, 36), np.float16)
        sp[:, :, :35, :35] = XG[:, 4 * k:4 * k + PL + 1]
        corners = [sp[:, ed:ed + PL, eh:eh + 35, ew:ew + 35]
                   for ed in (0, 1) for eh in (0, 1) for ew in (0, 1)]
        tbl = np.stack(corners, 0).transpose(2, 3, 4, 0, 1).reshape(TROWS, 256)
        tbl = np.ascontiguousarray(tbl)
        dclip = np.zeros((128, 2), np.float32)
        dclip[:, 0] = 0.0 - dlo
        dclip[:, 1] = 33.0 - dlo
        in_maps.append({
            "xr": xr,
            "tbl": tbl,
            "pc": pc_t,
            "dclip": dclip,
            "wp": wp_l,
            "wd": wd_l,
            "sel": sel,
        })
    return in_maps


def kernel(x, w_p, b_p, w_d):
    if "nc" not in _PROGRAM_CACHE:
        _PROGRAM_CACHE["nc"] = _build_program()
    nc = _PROGRAM_CACHE["nc"]
    in_maps = _host_prep(x, w_p, b_p, w_d)
    res = run_bass_kernel_spmd(nc, in_maps, list(range(NCORES))).results
    out = np.empty((1, O, 32, 32, 32), np.float32)
    for k in range(NCORES):
        out[0, :, 4 * k:4 * k + 4] = res[k]["out_sl"].reshape(O, DSH, 32, 32)
    return out


# revision 12
# speedup vs baseline: 1.3595x; 1.3595x over previous
"""Deformable 3D conv (offset-predicting conv + trilinear-sampled 3x3x3 deform conv)
on 8 TRN2 NeuronCores.

Strategy (v4): shard the output D axis (4 planes/core). Per core:
  1. Offset conv as 9 K=96 fp16 matmuls per v-tile (3 pre-shifted input copies
     fold the kw taps into the K dim); split 4+4 vtiles so quarter-0 p-pipeline
     starts early.
  2. p-pipeline on DVE in 4 quarters, software-pipelined one quarter ahead of
     the gather loop: clip, floor, fracs, indices, 8 trilinear corner weights.
  3. Corner-block table (512B row per padded voxel: 8 corners x 32ch fp16)
     built on HOST, passed as a DRAM input.
  4. Wrapped int16 index tensor built via PE select-matmuls (partition
     16g+q -> q with 8x replication) + strided DVE PSUM->SBUF casts.
  5. Batched gather: 32 dma_gather calls (3456 rows each); Q7 descriptor
     generation on Pool is the critical resource. Enlarged SWDGE carveout to
     keep the descriptor ring from stalling generation.
  6. Weighted corner sum on DVE (one fp16 mult + 3 tree adds per chunk);
     weight replication across channels on the scalar (ACT) engine.
  7. Contraction over (n, c) as 7 accumulated PE matmuls -> out[64, 128]/chunk.
"""
import os
import sys

for _p in ('/opt/trn_rl_repo', '/root/.axon_site/_ro/trn_rl_repo'):
    if os.path.isdir(_p) and _p not in sys.path:
        sys.path.insert(0, _p)

import numpy as np
import ml_dtypes  # noqa

import concourse.bass as bass
import concourse.mybir as mybir
import concourse.tile as tile
from concourse import bacc
from concourse.bass_utils import run_bass_kernel_spmd
from concourse.masks import make_identity

F32 = mybir.dt.float32
F16 = mybir.dt.float16
I32 = mybir.dt.int32
I16 = mybir.dt.int16
AL = mybir.AluOpType

# ---------------- problem constants ----------------
C = 32          # input channels
O = 64          # output channels
NN = 27         # kernel sample points
NCORES = 8
DSH = 4         # output d-planes per core
V = DSH * 32 * 32   # voxels per core = 4096
P35 = 35
PL = 16         # extended d-planes per core
PLSZ = P35 * P35    # 1225
XE_ROWS = PL * PLSZ  # 19600
TROWS = XE_ROWS      # table rows (one per padded voxel)
XR_FREE = 19616      # shifted-copy free size (19600 + shift pad)
NVC = 32             # v-chunks of 128
NQ = 4               # p-pipeline quarters
QC = NVC // NQ       # chunks per quarter = 8
GI = NN * 128        # indices per gather (one chunk) = 3456

_PROGRAM_CACHE = {}


def _build_program():
    nc = bacc.Bacc("TRN2", target_bir_lowering=False, debug=False,
                   dynamic_dma_scratch_size=16384,
                   num_swdge_queues=2)

    xr_d = nc.dram_tensor("xr", [96, XR_FREE], F16, kind="ExternalInput").ap()
    tbl_d = nc.dram_tensor("tbl", [TROWS, 256], F16, kind="ExternalInput").ap()
    pc_d = nc.dram_tensor("pc", [128, NVC * 96], F32, kind="ExternalInput").ap()
    dclip_d = nc.dram_tensor("dclip", [128, 2], F32, kind="ExternalInput").ap()
    wp_d = nc.dram_tensor("wp", [96, 9 * 96], F16, kind="ExternalInput").ap()
    wd_d = nc.dram_tensor("wd", [128, 7 * O], F16, kind="ExternalInput").ap()
    sel_d = nc.dram_tensor("sel", [128, 8 * 128], F32, kind="ExternalInput").ap()
    out_d = nc.dram_tensor("out_sl", [O, V], F32, kind="ExternalOutput").ap()

    with tile.TileContext(nc) as tc:
        with tc.tile_pool(name="const", bufs=1) as cpool, \
             tc.tile_pool(name="idxps", bufs=2, space="PSUM") as ips:
            ident = cpool.tile([128, 128], F32)
            make_identity(nc, ident[:])
            wd_sb = cpool.tile([128, 7 * O], F16)
            nc.sync.dma_start(wd_sb[:], wd_d)
            sel_sb = cpool.tile([128, 8 * 128], F32)
            nc.sync.dma_start(sel_sb[:], sel_d)
            # wrapped int16 gather indices: [q+16r, s*216 + n*8 + g] =
            # idx(voxel s*128+16g+q, tap n), replicated over r
            idxw = cpool.tile([128, NVC * 216], I16)
            # trilinear corner weights: [p, (vc*27) * 8]
            wt8 = cpool.tile([128, NVC * 216], F16)
            # persistent p-pipeline tensors
            p_t = cpool.tile([128, NVC * 96], F32)
            q0i = cpool.tile([128, NVC * 96], I32)
            q0f = cpool.tile([128, NVC * 96], F32)
            fd = cpool.tile([128, NVC * 27], F16)
            fh = cpool.tile([128, NVC * 27], F16)
            fw = cpool.tile([128, NVC * 27], F16)
            idxf = cpool.tile([128, NVC * 27], F32)
            wdd = cpool.tile([128, NVC * 27 * 2], F16)
            whh = cpool.tile([128, NVC * 27 * 2], F16)
            www = cpool.tile([128, NVC * 27 * 2], F16)
            t4 = cpool.tile([128, NVC * 27 * 4], F16)
            dclip_sb = cpool.tile([128, 2], F32)
            nc.sync.dma_start(dclip_sb[:], dclip_d)

            pvq = p_t[:].rearrange("p (v x) -> p v x", x=96)
            qvq = q0f[:].rearrange("p (v x) -> p v x", x=96)
            qiq = q0i[:].rearrange("p (v x) -> p v x", x=96)
            iwv = idxw[:].rearrange("p (s n g) -> p s n g", n=NN, g=8)

            def pipe_quarter(q):
                s0, s1 = q * QC, (q + 1) * QC
                dv = pvq[:, s0:s1, 0:27]
                hwv = pvq[:, s0:s1, 32:91]
                nc.vector.scalar_tensor_tensor(
                    out=dv, in0=dv, scalar=dclip_sb[:, 0:1],
                    in1=dclip_sb[:, 1:2].rearrange(
                        "p (a b) -> p a b", b=1).to_broadcast((128, QC, 27)),
                    op0=AL.max, op1=AL.min)
                nc.vector.tensor_scalar(
                    out=hwv, in0=hwv, scalar1=0.0, scalar2=33.0,
                    op0=AL.max, op1=AL.min)

                cl, ch_ = s0 * 96, s1 * 96
                nc.vector.tensor_copy(q0i[:, cl:ch_], p_t[:, cl:ch_])
                nc.vector.tensor_copy(q0f[:, cl:ch_], q0i[:, cl:ch_])
                # guard against round-to-nearest casts: q0f -= (q0f > p)
                # (reuse q0i's storage for the guard mask)
                nc.vector.tensor_tensor(
                    out=qiq[:, s0:s1, :].bitcast(F32),
                    in0=qvq[:, s0:s1, :], in1=pvq[:, s0:s1, :], op=AL.is_gt)
                nc.vector.tensor_sub(
                    qvq[:, s0:s1, :], qvq[:, s0:s1, :],
                    qiq[:, s0:s1, :].bitcast(F32))

                # fracs (fp16) per axis
                fl, fh_ = s0 * 27, s1 * 27
                fdv = fd[:, fl:fh_].rearrange("p (v x) -> p v x", x=27)
                fhv = fh[:, fl:fh_].rearrange("p (v x) -> p v x", x=27)
                fwv = fw[:, fl:fh_].rearrange("p (v x) -> p v x", x=27)
                nc.vector.tensor_sub(fdv, pvq[:, s0:s1, 0:27], qvq[:, s0:s1, 0:27])
                nc.vector.tensor_sub(fhv, pvq[:, s0:s1, 32:59], qvq[:, s0:s1, 32:59])
                nc.vector.tensor_sub(fwv, pvq[:, s0:s1, 64:91], qvq[:, s0:s1, 64:91])

                # d-axis safety clamp to [0, 14]
                q0dv = qvq[:, s0:s1, 0:27]
                nc.vector.tensor_scalar(
                    out=q0dv, in0=q0dv, scalar1=0.0, scalar2=14.0,
                    op0=AL.max, op1=AL.min)

                # idx = (q0d*35 + q0h)*35 + q0w
                iv = idxf[:, fl:fh_].rearrange("p (v x) -> p v x", x=27)
                nc.vector.scalar_tensor_tensor(
                    out=iv, in0=q0dv, scalar=35.0, in1=qvq[:, s0:s1, 32:59],
                    op0=AL.mult, op1=AL.add)
                nc.vector.scalar_tensor_tensor(
                    out=iv, in0=iv, scalar=35.0, in1=qvq[:, s0:s1, 64:91],
                    op0=AL.mult, op1=AL.add)

                # corner weights wt8 for this quarter
                for pair_t, frac_t in ((wdd, fd), (whh, fh), (www, fw)):
                    pvw = pair_t[:, fl * 2:fh_ * 2].rearrange(
                        "p (i e) -> p i e", e=2)
                    fv1 = frac_t[:, fl:fh_].rearrange("p (i o) -> p i o", o=1)
                    nc.vector.tensor_scalar(
                        out=pvw[:, :, 0:1], in0=fv1, scalar1=-1.0, scalar2=1.0,
                        op0=AL.mult, op1=AL.add)
                    nc.vector.tensor_copy(pvw[:, :, 1:2], fv1)
                t4v = t4[:, fl * 4:fh_ * 4].rearrange(
                    "p (i a b) -> p i a b", a=2, b=2)
                nc.vector.tensor_tensor(
                    out=t4v,
                    in0=whh[:, fl * 2:fh_ * 2].rearrange(
                        "p (i a b) -> p i a b", a=2, b=1
                    ).to_broadcast((128, QC * 27, 2, 2)),
                    in1=www[:, fl * 2:fh_ * 2].rearrange(
                        "p (i a b) -> p i a b", a=1, b=2
                    ).to_broadcast((128, QC * 27, 2, 2)),
                    op=AL.mult)
                w8v = wt8[:, fl * 8:fh_ * 8].rearrange(
                    "p (i a b) -> p i a b", a=2, b=4)
                nc.vector.tensor_tensor(
                    out=w8v,
                    in0=wdd[:, fl * 2:fh_ * 2].rearrange(
                        "p (i a b) -> p i a b", a=2, b=1
                    ).to_broadcast((128, QC * 27, 2, 4)),
                    in1=t4[:, fl * 4:fh_ * 4].rearrange(
                        "p (i a b) -> p i a b", a=1, b=4
                    ).to_broadcast((128, QC * 27, 2, 4)),
                    op=AL.mult)

                # wrapped idx via PE select-matmuls + strided cast copies
                for g in range(8):
                    pidx = ips.tile([128, QC * 27], F32, tag="idxps")
                    nc.tensor.matmul(
                        pidx[:, :],
                        lhsT=sel_sb[:, g * 128:(g + 1) * 128],
                        rhs=idxf[:, fl:fh_],
                        start=True, stop=True)
                    nc.vector.tensor_copy(
                        iwv[:, s0:s1, :, g],
                        pidx[:, :].rearrange("p (s n) -> p s n", n=NN))

            # ---------- front: offset conv + transpose ----------
            with tc.tile_pool(name="front", bufs=1) as fpool:
                xr_sb = fpool.tile([96, XR_FREE], F16)
                nc.sync.dma_start(xr_sb[:], xr_d)
                wp_sb = fpool.tile([96, 9 * 96], F16)
                nc.sync.dma_start(wp_sb[:], wp_d)
                pc_sb = fpool.tile([128, NVC * 96], F32)
                nc.sync.dma_start(pc_sb[:], pc_d)
                off_sb = fpool.tile([96, V], F32)

                def conv_vtiles(vts, cps):
                    for vt in vts:
                        dl, hh = vt // 2, vt % 2
                        psc = cps.tile([96, 512], F32, tag="convps")
                        for gid in range(9):
                            kd, kh = gid // 3, gid % 3
                            b0 = (dl + kd + 5) * PLSZ + (hh * 16 + kh) * P35
                            rhs = xr_sb[:, b0:b0 + 16 * P35].rearrange(
                                "p (a b) -> p a b", b=P35)[:, :, 0:32]
                            nc.tensor.matmul(
                                psc[:, :],
                                lhsT=wp_sb[:, gid * 96:(gid + 1) * 96],
                                rhs=rhs,
                                start=(gid == 0),
                                stop=(gid == 8),
                            )
                        nc.scalar.copy(off_sb[:, vt * 512:(vt + 1) * 512], psc[:, :])

                def transpose_chunks(chs, tps):
                    for ch in chs:
                        ptp = tps.tile([128, 96], F32, tag="trps")
                        nc.tensor.transpose(
                            ptp[:, :],
                            off_sb[:, ch * 128:(ch + 1) * 128],
                            ident[0:96, 0:96],
                        )
                        nc.vector.tensor_add(
                            p_t[:, ch * 96:(ch + 1) * 96], ptp[:, :],
                            pc_sb[:, ch * 96:(ch + 1) * 96])

                with tc.tile_pool(name="convps", bufs=2, space="PSUM") as cps, \
                     tc.tile_pool(name="trps", bufs=2, space="PSUM") as tps:
                    conv_vtiles(range(0, 4), cps)
                    transpose_chunks(range(0, 8), tps)
                    pipe_quarter(0)
                    conv_vtiles(range(4, 8), cps)
                    transpose_chunks(range(8, 32), tps)
                    pipe_quarter(1)

            # ---------- gather + lerp + contract, one quarter ahead ----------
            with (
                tc.tile_pool(name="gat", bufs=4) as gpool,
                tc.tile_pool(name="wrep", bufs=2) as wpool,
                tc.tile_pool(name="accp", bufs=4) as apool,
                tc.tile_pool(name="acctp", bufs=4) as t2pool,
                tc.tile_pool(name="ops", bufs=4, space="PSUM") as ops,
                tc.tile_pool(name="outp", bufs=4) as opool,
            ):
                for q in range(NQ):
                    if q + 2 < NQ:
                        pipe_quarter(q + 2)
                    for s in range(q * QC, (q + 1) * QC):
                        rt = gpool.tile([128, NN * 256], F16, tag="rt")
                        nc.gpsimd.dma_gather(
                            rt[:].rearrange("p (i x) -> p i x", x=256),
                            tbl_d,
                            idxw[:, s * 216:(s + 1) * 216],
                            GI,
                            GI,
                            256,
                            single_packet=False,
                            queue_num=s % 2,
                        )
                        wt8r = wpool.tile([128, NN * 256], F16, tag="wt8r")
                        nc.scalar.copy(
                            wt8r[:].rearrange("p (i e c) -> p i e c", e=8, c=32),
                            wt8[:, s * 216:(s + 1) * 216].rearrange(
                                "p (i e o) -> p i e o", e=8, o=1
                            ).to_broadcast((128, NN, 8, 32)))

                        # weighted corner sum
                        nc.vector.tensor_tensor(
                            out=rt[:], in0=rt[:], in1=wt8r[:], op=AL.mult)
                        rv = rt[:].rearrange("p (i x) -> p i x", x=256)
                        nc.vector.tensor_add(
                            rv[:, :, 0:128], rv[:, :, 0:128], rv[:, :, 128:256])
                        nc.vector.tensor_add(
                            rv[:, :, 0:64], rv[:, :, 0:64], rv[:, :, 64:128])
                        acc = apool.tile([128, 896], F16, tag="acc")
                        nc.vector.tensor_tensor(
                            out=acc[:, 0:864].rearrange("p (n c) -> p n c", c=32),
                            in0=rv[:, :, 0:32],
                            in1=rv[:, :, 32:64],
                            op=AL.add)
                        nc.vector.memset(acc[:, 864:896], 0.0)

                        # transpose + contract + write out
                        acct = t2pool.tile([128, 7, 128], F16, tag="acct")
                        nc.sync.dma_start_transpose(out=acct[:], in_=acc[:])
                        pso = ops.tile([64, 128], F32, tag="pso")
                        for kt in range(7):
                            nc.tensor.matmul(
                                pso[:, :],
                                lhsT=wd_sb[:, kt * O:(kt + 1) * O],
                                rhs=acct[:, kt, :],
                                start=(kt == 0), stop=(kt == 6))
                        osb = opool.tile([64, 128], F32, tag="osb")
                        nc.scalar.copy(osb[:], pso[:, :])
                        nc.sync.dma_start(
                            out=out_d[:, s * 128:(s + 1) * 128], in_=osb[:])

    nc.compile()
    return nc


def _host_prep(x, w_p, b_p, w_d):
    """Build per-core input maps."""
    x = np.asarray(x, np.float32)
    w_p = np.asarray(w_p, np.float32)
    b_p = np.asarray(b_p, np.float32)
    w_d = np.asarray(w_d, np.float32)

    # global padded/extended volume, channel-first, fp16:
    # XG[c, g, h', w'] with g = xp_plane + 5 (xp planes -5..39), h', w' in [0,35)
    XG = np.zeros((C, 45, P35, P35), np.float16)
    XG[:, 6:38, 1:33, 1:33] = x[0].astype(np.float16)

    # pc (shared): [128, 32*96] f32
    v = np.arange(V)
    dl, hh, wl = v >> 10, (v >> 5) & 31, v & 31
    r = np.array([-1.0, 0.0, 1.0], np.float32)
    pn_d, pn_h, pn_w = np.meshgrid(r, r, r, indexing='ij')
    pn = np.stack([pn_d.ravel(), pn_h.ravel(), pn_w.ravel()])  # (3, 27)
    pc = np.zeros((V, 96), np.float32)
    pc[:, 0:27] = (dl[:, None] + 6.0) + pn[0][None, :] + b_p[None, 0:27]
    pc[:, 32:59] = (hh[:, None] + 1.0) + pn[1][None, :] + b_p[None, 27:54]
    pc[:, 64:91] = (wl[:, None] + 1.0) + pn[2][None, :] + b_p[None, 54:81]
    pc_t = pc.reshape(NVC, 128, 96).transpose(1, 0, 2).reshape(128, NVC * 96)
    pc_t = np.ascontiguousarray(pc_t, np.float32)

    # wp lhsT: [96, 9*96] fp16; K-row (j, c) = tap (kd, kh, kw=j)
    wp_l = np.zeros((96, 9 * 96), np.float16)
    colmap = np.full(96, -1, np.int64)
    colmap[0:27] = np.arange(27)
    colmap[32:59] = 27 + np.arange(27)
    colmap[64:91] = 54 + np.arange(27)
    for gid in range(9):
        kd, kh = gid // 3, gid % 3
        for m in range(96):
            ch = colmap[m]
            if ch < 0:
                continue
            for j in range(3):
                wp_l[32 * j:32 * (j + 1), gid * 96 + m] = w_p[ch, :, kd, kh, j]

    # wd lhsT: [128, 7*64] fp16 (K-row (g, pk): n = 4g + pk//32, c = pk%32)
    wd_l = np.zeros((128, 7 * O), np.float16)
    for g in range(7):
        for pk in range(128):
            n = 4 * g + pk // 32
            if n >= NN:
                continue
            wd_l[pk, g * O:(g + 1) * O] = w_d[:, pk % 32, n // 9, (n // 3) % 3, n % 3]

    # sel: [128, 8*128] f32; sel[p, g*128 + m] = (p == 16g + m%16)
    sel = np.zeros((128, 8 * 128), np.float32)
    for g in range(8):
        for m in range(128):
            sel[16 * g + (m % 16), g * 128 + m] = 1.0

    in_maps = []
    for k in range(NCORES):
        dlo = 4 * k - 5
        slab = XG[:, 4 * k:4 * k + PL].reshape(C, XE_ROWS)
        xr = np.zeros((96, XR_FREE), np.float16)
        for j in range(3):
            xr[32 * j:32 * (j + 1), 0:XE_ROWS - j] = slab[:, j:]
        # corner-block table: row (d*35+h)*35+w -> [8 corners x 32 ch] fp16
        sp = np.zeros((C, PL + 1, 36, 36), np.float16)
        sp[:, :, :35, :35] = XG[:, 4 * k:4 * k + PL + 1]
        corners = [sp[:, ed:ed + PL, eh:eh + 35, ew:ew + 35]
                   for ed in (0, 1) for eh in (0, 1) for ew in (0, 1)]
        tbl = np.stack(corners, 0).transpose(2, 3, 4, 0, 1).reshape(TROWS, 256)
        tbl = np.ascontiguousarray(tbl)
        dclip = np.zeros((128, 2), np.float32)
        dclip[:, 0] = 0.0 - dlo
        dclip[:, 1] = 33.0 - dlo
        in_maps.append({
            "xr": xr,
            "tbl": tbl,
            "pc": pc_t,
            "dclip": dclip,
            "wp": wp_l,
            "wd": wd_l,
            "sel": sel,
        })
    return in_maps


def kernel(x, w_p, b_p, w_d):
    if "nc" not in _PROGRAM_CACHE:
        _PROGRAM_CACHE["nc"] = _build_program()
    nc = _PROGRAM_CACHE["nc"]
    in_maps = _host_prep(x, w_p, b_p, w_d)
    res = run_bass_kernel_spmd(nc, in_maps, list(range(NCORES))).results
    out = np.empty((1, O, 32, 32, 32), np.float32)
    for k in range(NCORES):
        out[0, :, 4 * k:4 * k + 4] = res[k]["out_sl"].reshape(O, DSH, 32, 32)
    return out


# revision 13
# speedup vs baseline: 1.3882x; 1.0211x over previous
"""Deformable 3D conv (offset-predicting conv + trilinear-sampled 3x3x3 deform conv)
on 8 TRN2 NeuronCores.

Strategy (v4): shard the output D axis (4 planes/core). Per core:
  1. Offset conv as 9 K=96 fp16 matmuls per v-tile (3 pre-shifted input copies
     fold the kw taps into the K dim); split 4+4 vtiles so quarter-0 p-pipeline
     starts early.
  2. p-pipeline on DVE in 4 quarters, software-pipelined one quarter ahead of
     the gather loop: clip, floor, fracs, indices, 8 trilinear corner weights.
  3. Corner-block table (512B row per padded voxel: 8 corners x 32ch fp16)
     built on HOST, passed as a DRAM input.
  4. Wrapped int16 index tensor built via PE select-matmuls (partition
     16g+q -> q with 8x replication) + strided DVE PSUM->SBUF casts.
  5. Batched gather: 32 dma_gather calls (3456 rows each); Q7 descriptor
     generation on Pool is the critical resource. Enlarged SWDGE carveout to
     keep the descriptor ring from stalling generation.
  6. Weighted corner sum on DVE (one fp16 mult + 3 tree adds per chunk);
     weight replication across channels on the scalar (ACT) engine.
  7. Contraction over (n, c) as 7 accumulated PE matmuls -> out[64, 128]/chunk.
"""
import os
import sys

for _p in ('/opt/trn_rl_repo', '/root/.axon_site/_ro/trn_rl_repo'):
    if os.path.isdir(_p) and _p not in sys.path:
        sys.path.insert(0, _p)

import numpy as np
import ml_dtypes  # noqa

import concourse.bass as bass
import concourse.mybir as mybir
import concourse.tile as tile
from concourse import bacc
from concourse.bass_utils import run_bass_kernel_spmd
from concourse.masks import make_identity

F32 = mybir.dt.float32
F16 = mybir.dt.float16
I32 = mybir.dt.int32
I16 = mybir.dt.int16
AL = mybir.AluOpType

# ---------------- problem constants ----------------
C = 32          # input channels
O = 64          # output channels
NN = 27         # kernel sample points
NCORES = 8
DSH = 4         # output d-planes per core
V = DSH * 32 * 32   # voxels per core = 4096
P35 = 35
PL = 16         # extended d-planes per core
PLSZ = P35 * P35    # 1225
XE_ROWS = PL * PLSZ  # 19600
TROWS = XE_ROWS      # table rows (one per padded voxel)
XR_FREE = 19616      # shifted-copy free size (19600 + shift pad)
NVC = 32             # v-chunks of 128
NQ = 4               # p-pipeline quarters
QC = NVC // NQ       # chunks per quarter = 8
GI = NN * 128        # indices per gather (one chunk) = 3456

_PROGRAM_CACHE = {}


def _build_program():
    nc = bacc.Bacc("TRN2", target_bir_lowering=False, debug=False,
                   dynamic_dma_scratch_size=16384,
                   num_swdge_queues=3)

    xr_d = nc.dram_tensor("xr", [96, XR_FREE], F16, kind="ExternalInput").ap()
    tbl_d = nc.dram_tensor("tbl", [TROWS, 256], F16, kind="ExternalInput").ap()
    pc_d = nc.dram_tensor("pc", [128, NVC * 96], F32, kind="ExternalInput").ap()
    dclip_d = nc.dram_tensor("dclip", [128, 2], F32, kind="ExternalInput").ap()
    wp_d = nc.dram_tensor("wp", [96, 9 * 96], F16, kind="ExternalInput").ap()
    wd_d = nc.dram_tensor("wd", [128, 7 * O], F16, kind="ExternalInput").ap()
    sel_d = nc.dram_tensor("sel", [128, 8 * 128], F32, kind="ExternalInput").ap()
    out_d = nc.dram_tensor("out_sl", [O, V], F32, kind="ExternalOutput").ap()

    with tile.TileContext(nc) as tc:
        with tc.tile_pool(name="const", bufs=1) as cpool, \
             tc.tile_pool(name="idxps", bufs=2, space="PSUM") as ips:
            ident = cpool.tile([128, 128], F32)
            make_identity(nc, ident[:])
            wd_sb = cpool.tile([128, 7 * O], F16)
            nc.sync.dma_start(wd_sb[:], wd_d)
            sel_sb = cpool.tile([128, 8 * 128], F32)
            nc.sync.dma_start(sel_sb[:], sel_d)
            # wrapped int16 gather indices: [q+16r, s*216 + n*8 + g] =
            # idx(voxel s*128+16g+q, tap n), replicated over r
            idxw = cpool.tile([128, NVC * 216], I16)
            # trilinear corner weights: [p, (vc*27) * 8]
            wt8 = cpool.tile([128, NVC * 216], F16)
            # persistent p-pipeline tensors
            p_t = cpool.tile([128, NVC * 96], F32)
            q0i = cpool.tile([128, NVC * 96], I32)
            q0f = cpool.tile([128, NVC * 96], F32)
            fd = cpool.tile([128, NVC * 27], F16)
            fh = cpool.tile([128, NVC * 27], F16)
            fw = cpool.tile([128, NVC * 27], F16)
            idxf = cpool.tile([128, NVC * 27], F32)
            wdd = cpool.tile([128, NVC * 27 * 2], F16)
            whh = cpool.tile([128, NVC * 27 * 2], F16)
            www = cpool.tile([128, NVC * 27 * 2], F16)
            t4 = cpool.tile([128, NVC * 27 * 4], F16)
            dclip_sb = cpool.tile([128, 2], F32)
            nc.sync.dma_start(dclip_sb[:], dclip_d)

            pvq = p_t[:].rearrange("p (v x) -> p v x", x=96)
            qvq = q0f[:].rearrange("p (v x) -> p v x", x=96)
            qiq = q0i[:].rearrange("p (v x) -> p v x", x=96)
            iwv = idxw[:].rearrange("p (s n g) -> p s n g", n=NN, g=8)

            def pipe_quarter(q):
                s0, s1 = q * QC, (q + 1) * QC
                dv = pvq[:, s0:s1, 0:27]
                hwv = pvq[:, s0:s1, 32:91]
                nc.vector.scalar_tensor_tensor(
                    out=dv, in0=dv, scalar=dclip_sb[:, 0:1],
                    in1=dclip_sb[:, 1:2].rearrange(
                        "p (a b) -> p a b", b=1).to_broadcast((128, QC, 27)),
                    op0=AL.max, op1=AL.min)
                nc.vector.tensor_scalar(
                    out=hwv, in0=hwv, scalar1=0.0, scalar2=33.0,
                    op0=AL.max, op1=AL.min)

                cl, ch_ = s0 * 96, s1 * 96
                nc.vector.tensor_copy(q0i[:, cl:ch_], p_t[:, cl:ch_])
                nc.vector.tensor_copy(q0f[:, cl:ch_], q0i[:, cl:ch_])
                # guard against round-to-nearest casts: q0f -= (q0f > p)
                # (reuse q0i's storage for the guard mask)
                nc.vector.tensor_tensor(
                    out=qiq[:, s0:s1, :].bitcast(F32),
                    in0=qvq[:, s0:s1, :], in1=pvq[:, s0:s1, :], op=AL.is_gt)
                nc.vector.tensor_sub(
                    qvq[:, s0:s1, :], qvq[:, s0:s1, :],
                    qiq[:, s0:s1, :].bitcast(F32))

                # fracs (fp16) per axis
                fl, fh_ = s0 * 27, s1 * 27
                fdv = fd[:, fl:fh_].rearrange("p (v x) -> p v x", x=27)
                fhv = fh[:, fl:fh_].rearrange("p (v x) -> p v x", x=27)
                fwv = fw[:, fl:fh_].rearrange("p (v x) -> p v x", x=27)
                nc.vector.tensor_sub(fdv, pvq[:, s0:s1, 0:27], qvq[:, s0:s1, 0:27])
                nc.vector.tensor_sub(fhv, pvq[:, s0:s1, 32:59], qvq[:, s0:s1, 32:59])
                nc.vector.tensor_sub(fwv, pvq[:, s0:s1, 64:91], qvq[:, s0:s1, 64:91])

                # d-axis safety clamp to [0, 14]
                q0dv = qvq[:, s0:s1, 0:27]
                nc.vector.tensor_scalar(
                    out=q0dv, in0=q0dv, scalar1=0.0, scalar2=14.0,
                    op0=AL.max, op1=AL.min)

                # idx = (q0d*35 + q0h)*35 + q0w
                iv = idxf[:, fl:fh_].rearrange("p (v x) -> p v x", x=27)
                nc.vector.scalar_tensor_tensor(
                    out=iv, in0=q0dv, scalar=35.0, in1=qvq[:, s0:s1, 32:59],
                    op0=AL.mult, op1=AL.add)
                nc.vector.scalar_tensor_tensor(
                    out=iv, in0=iv, scalar=35.0, in1=qvq[:, s0:s1, 64:91],
                    op0=AL.mult, op1=AL.add)

                # corner weights wt8 for this quarter
                for pair_t, frac_t in ((wdd, fd), (whh, fh), (www, fw)):
                    pvw = pair_t[:, fl * 2:fh_ * 2].rearrange(
                        "p (i e) -> p i e", e=2)
                    fv1 = frac_t[:, fl:fh_].rearrange("p (i o) -> p i o", o=1)
                    nc.vector.tensor_scalar(
                        out=pvw[:, :, 0:1], in0=fv1, scalar1=-1.0, scalar2=1.0,
                        op0=AL.mult, op1=AL.add)
                    nc.vector.tensor_copy(pvw[:, :, 1:2], fv1)
                t4v = t4[:, fl * 4:fh_ * 4].rearrange(
                    "p (i a b) -> p i a b", a=2, b=2)
                nc.vector.tensor_tensor(
                    out=t4v,
                    in0=whh[:, fl * 2:fh_ * 2].rearrange(
                        "p (i a b) -> p i a b", a=2, b=1
                    ).to_broadcast((128, QC * 27, 2, 2)),
                    in1=www[:, fl * 2:fh_ * 2].rearrange(
                        "p (i a b) -> p i a b", a=1, b=2
                    ).to_broadcast((128, QC * 27, 2, 2)),
                    op=AL.mult)
                w8v = wt8[:, fl * 8:fh_ * 8].rearrange(
                    "p (i a b) -> p i a b", a=2, b=4)
                nc.vector.tensor_tensor(
                    out=w8v,
                    in0=wdd[:, fl * 2:fh_ * 2].rearrange(
                        "p (i a b) -> p i a b", a=2, b=1
                    ).to_broadcast((128, QC * 27, 2, 4)),
                    in1=t4[:, fl * 4:fh_ * 4].rearrange(
                        "p (i a b) -> p i a b", a=1, b=4
                    ).to_broadcast((128, QC * 27, 2, 4)),
                    op=AL.mult)

                # wrapped idx via PE select-matmuls + strided cast copies
                for g in range(8):
                    pidx = ips.tile([128, QC * 27], F32, tag="idxps")
                    nc.tensor.matmul(
                        pidx[:, :],
                        lhsT=sel_sb[:, g * 128:(g + 1) * 128],
                        rhs=idxf[:, fl:fh_],
                        start=True, stop=True)
                    nc.vector.tensor_copy(
                        iwv[:, s0:s1, :, g],
                        pidx[:, :].rearrange("p (s n) -> p s n", n=NN))

            # ---------- front: offset conv + transpose ----------
            with tc.tile_pool(name="front", bufs=1) as fpool:
                xr_sb = fpool.tile([96, XR_FREE], F16)
                nc.sync.dma_start(xr_sb[:], xr_d)
                wp_sb = fpool.tile([96, 9 * 96], F16)
                nc.sync.dma_start(wp_sb[:], wp_d)
                pc_sb = fpool.tile([128, NVC * 96], F32)
                nc.sync.dma_start(pc_sb[:], pc_d)
                off_sb = fpool.tile([96, V], F32)

                def conv_vtiles(vts, cps):
                    for vt in vts:
                        dl, hh = vt // 2, vt % 2
                        psc = cps.tile([96, 512], F32, tag="convps")
                        for gid in range(9):
                            kd, kh = gid // 3, gid % 3
                            b0 = (dl + kd + 5) * PLSZ + (hh * 16 + kh) * P35
                            rhs = xr_sb[:, b0:b0 + 16 * P35].rearrange(
                                "p (a b) -> p a b", b=P35)[:, :, 0:32]
                            nc.tensor.matmul(
                                psc[:, :],
                                lhsT=wp_sb[:, gid * 96:(gid + 1) * 96],
                                rhs=rhs,
                                start=(gid == 0),
                                stop=(gid == 8),
                            )
                        nc.scalar.copy(off_sb[:, vt * 512:(vt + 1) * 512], psc[:, :])

                def transpose_chunks(chs, tps):
                    for ch in chs:
                        ptp = tps.tile([128, 96], F32, tag="trps")
                        nc.tensor.transpose(
                            ptp[:, :],
                            off_sb[:, ch * 128:(ch + 1) * 128],
                            ident[0:96, 0:96],
                        )
                        nc.vector.tensor_add(
                            p_t[:, ch * 96:(ch + 1) * 96], ptp[:, :],
                            pc_sb[:, ch * 96:(ch + 1) * 96])

                with tc.tile_pool(name="convps", bufs=2, space="PSUM") as cps, \
                     tc.tile_pool(name="trps", bufs=2, space="PSUM") as tps:
                    conv_vtiles(range(0, 4), cps)
                    transpose_chunks(range(0, 8), tps)
                    pipe_quarter(0)
                    conv_vtiles(range(4, 8), cps)
                    transpose_chunks(range(8, 32), tps)
                    pipe_quarter(1)

            # ---------- gather + lerp + contract, one quarter ahead ----------
            with (
                tc.tile_pool(name="gat", bufs=4) as gpool,
                tc.tile_pool(name="wrep", bufs=2) as wpool,
                tc.tile_pool(name="accp", bufs=4) as apool,
                tc.tile_pool(name="acctp", bufs=4) as t2pool,
                tc.tile_pool(name="ops", bufs=4, space="PSUM") as ops,
                tc.tile_pool(name="outp", bufs=4) as opool,
            ):
                for q in range(NQ):
                    if q + 2 < NQ:
                        pipe_quarter(q + 2)
                    for s in range(q * QC, (q + 1) * QC):
                        rt = gpool.tile([128, NN * 256], F16, tag="rt")
                        nc.gpsimd.dma_gather(
                            rt[:].rearrange("p (i x) -> p i x", x=256),
                            tbl_d,
                            idxw[:, s * 216:(s + 1) * 216],
                            GI,
                            GI,
                            256,
                            single_packet=False,
                            queue_num=s % 3,
                        )
                        wt8r = wpool.tile([128, NN * 256], F16, tag="wt8r")
                        nc.scalar.copy(
                            wt8r[:].rearrange("p (i e c) -> p i e c", e=8, c=32),
                            wt8[:, s * 216:(s + 1) * 216].rearrange(
                                "p (i e o) -> p i e o", e=8, o=1
                            ).to_broadcast((128, NN, 8, 32)))

                        # weighted corner sum
                        nc.vector.tensor_tensor(
                            out=rt[:], in0=rt[:], in1=wt8r[:], op=AL.mult)
                        rv = rt[:].rearrange("p (i x) -> p i x", x=256)
                        nc.vector.tensor_add(
                            rv[:, :, 0:128], rv[:, :, 0:128], rv[:, :, 128:256])
                        nc.vector.tensor_add(
                            rv[:, :, 0:64], rv[:, :, 0:64], rv[:, :, 64:128])
                        acc = apool.tile([128, 896], F16, tag="acc")
                        nc.vector.tensor_tensor(
                            out=acc[:, 0:864].rearrange("p (n c) -> p n c", c=32),
                            in0=rv[:, :, 0:32],
                            in1=rv[:, :, 32:64],
                            op=AL.add)
                        nc.vector.memset(acc[:, 864:896], 0.0)

                        # transpose + contract + write out
                        acct = t2pool.tile([128, 7, 128], F16, tag="acct")
                        nc.sync.dma_start_transpose(out=acct[:], in_=acc[:])
                        pso = ops.tile([64, 128], F32, tag="pso")
                        for kt in range(7):
                            nc.tensor.matmul(
                                pso[:, :],
                                lhsT=wd_sb[:, kt * O:(kt + 1) * O],
                                rhs=acct[:, kt, :],
                                start=(kt == 0), stop=(kt == 6))
                        osb = opool.tile([64, 128], F32, tag="osb")
                        nc.scalar.copy(osb[:], pso[:, :])
                        nc.sync.dma_start(
                            out=out_d[:, s * 128:(s + 1) * 128], in_=osb[:])

    nc.compile()
    return nc


def _host_prep(x, w_p, b_p, w_d):
    """Build per-core input maps."""
    x = np.asarray(x, np.float32)
    w_p = np.asarray(w_p, np.float32)
    b_p = np.asarray(b_p, np.float32)
    w_d = np.asarray(w_d, np.float32)

    # global padded/extended volume, channel-first, fp16:
    # XG[c, g, h', w'] with g = xp_plane + 5 (xp planes -5..39), h', w' in [0,35)
    XG = np.zeros((C, 45, P35, P35), np.float16)
    XG[:, 6:38, 1:33, 1:33] = x[0].astype(np.float16)

    # pc (shared): [128, 32*96] f32
    v = np.arange(V)
    dl, hh, wl = v >> 10, (v >> 5) & 31, v & 31
    r = np.array([-1.0, 0.0, 1.0], np.float32)
    pn_d, pn_h, pn_w = np.meshgrid(r, r, r, indexing='ij')
    pn = np.stack([pn_d.ravel(), pn_h.ravel(), pn_w.ravel()])  # (3, 27)
    pc = np.zeros((V, 96), np.float32)
    pc[:, 0:27] = (dl[:, None] + 6.0) + pn[0][None, :] + b_p[None, 0:27]
    pc[:, 32:59] = (hh[:, None] + 1.0) + pn[1][None, :] + b_p[None, 27:54]
    pc[:, 64:91] = (wl[:, None] + 1.0) + pn[2][None, :] + b_p[None, 54:81]
    pc_t = pc.reshape(NVC, 128, 96).transpose(1, 0, 2).reshape(128, NVC * 96)
    pc_t = np.ascontiguousarray(pc_t, np.float32)

    # wp lhsT: [96, 9*96] fp16; K-row (j, c) = tap (kd, kh, kw=j)
    wp_l = np.zeros((96, 9 * 96), np.float16)
    colmap = np.full(96, -1, np.int64)
    colmap[0:27] = np.arange(27)
    colmap[32:59] = 27 + np.arange(27)
    colmap[64:91] = 54 + np.arange(27)
    for gid in range(9):
        kd, kh = gid // 3, gid % 3
        for m in range(96):
            ch = colmap[m]
            if ch < 0:
                continue
            for j in range(3):
                wp_l[32 * j:32 * (j + 1), gid * 96 + m] = w_p[ch, :, kd, kh, j]

    # wd lhsT: [128, 7*64] fp16 (K-row (g, pk): n = 4g + pk//32, c = pk%32)
    wd_l = np.zeros((128, 7 * O), np.float16)
    for g in range(7):
        for pk in range(128):
            n = 4 * g + pk // 32
            if n >= NN:
                continue
            wd_l[pk, g * O:(g + 1) * O] = w_d[:, pk % 32, n // 9, (n // 3) % 3, n % 3]

    # sel: [128, 8*128] f32; sel[p, g*128 + m] = (p == 16g + m%16)
    sel = np.zeros((128, 8 * 128), np.float32)
    for g in range(8):
        for m in range(128):
            sel[16 * g + (m % 16), g * 128 + m] = 1.0

    in_maps = []
    for k in range(NCORES):
        dlo = 4 * k - 5
        slab = XG[:, 4 * k:4 * k + PL].reshape(C, XE_ROWS)
        xr = np.zeros((96, XR_FREE), np.float16)
        for j in range(3):
            xr[32 * j:32 * (j + 1), 0:XE_ROWS - j] = slab[:, j:]
        # corner-block table: row (d*35+h)*35+w -> [8 corners x 32 ch] fp16
        sp = np.zeros((C, PL + 1, 36, 36), np.float16)
        sp[:, :, :35, :35] = XG[:, 4 * k:4 * k + PL + 1]
        corners = [sp[:, ed:ed + PL, eh:eh + 35, ew:ew + 35]
                   for ed in (0, 1) for eh in (0, 1) for ew in (0, 1)]
        tbl = np.stack(corners, 0).transpose(2, 3, 4, 0, 1).reshape(TROWS, 256)
        tbl = np.ascontiguousarray(tbl)
        dclip = np.zeros((128, 2), np.float32)
        dclip[:, 0] = 0.0 - dlo
        dclip[:, 1] = 33.0 - dlo
        in_maps.append({
            "xr": xr,
            "tbl": tbl,
            "pc": pc_t,
            "dclip": dclip,
            "wp": wp_l,
            "wd": wd_l,
            "sel": sel,
        })
    return in_maps


def kernel(x, w_p, b_p, w_d):
    if "nc" not in _PROGRAM_CACHE:
        _PROGRAM_CACHE["nc"] = _build_program()
    nc = _PROGRAM_CACHE["nc"]
    in_maps = _host_prep(x, w_p, b_p, w_d)
    res = run_bass_kernel_spmd(nc, in_maps, list(range(NCORES))).results
    out = np.empty((1, O, 32, 32, 32), np.float32)
    for k in range(NCORES):
        out[0, :, 4 * k:4 * k + 4] = res[k]["out_sl"].reshape(O, DSH, 32, 32)
    return out
